# revision 4
# baseline (speedup 1.0000x reference)
"""Multi-head causal self-attention (q=k=v bug faithful) on 8 trn2 cores.

Sharding: 24 (batch, head) jobs -> 3 heads per core. Core c: batch c//4,
heads (c%4)*3 .. +3. Each core computes its heads' attention outputs and a
partial output-projection Z^T = sum_h O_h @ Wout_slice_h  (shape [768, 4096]).
Host: sum the 4 partials per batch, add nothing (bias folded in on one core),
transpose to [4096, 768].

Device algorithm per core (all matmuls fp32r, moving dim 512):
  1. Q^T[h] = (sqrt(s)*Wq_h) @ X^T   via 6 K-chunks of 128   (s = 1/sqrt(768))
  2. Q natural layout via PE transpose (plus a ones column for the softmax
     denominator)
  3. flash-style, i-groups of 512, j-blocks of 128 (causal-skipped):
       S^T[jb, i] = Q^T[:,jb-block].T @ Q^T[:,i-group]      (PSUM, 2 jb/bank-tile)
       P^T = exp(S^T)                 (ScalarE, PSUM->SBUF, span 1024)
       diag band masked by upper-tri 0/1 mask multiply (DVE)
       [O | denom]^T += [Q[jb]|1].T @ P^T                   (PSUM accum)
     normalize: O^T *= 1/denom (DVE recip + gpsimd partition_broadcast)
  4. Z^T[oc, i] = Wout_slice.T @ O_cat^T + bias  (6 out-chunks of 128)
"""

import os

import numpy as np

B, L, D, H, HS = 2, 4096, 768, 12, 64
NCORES = 8
HPC = 3  # heads per core
NIG = L // 512  # i-groups
SCALE = 1.0 / np.sqrt(np.float32(D))
SQS = np.sqrt(SCALE).astype(np.float32)  # folded into Wq (and undone in Wout)

_cached = {}


def _build_program():
    import concourse.bass as bass
    import concourse.mybir as mybir
    import concourse.tile as tile
    from concourse import bacc
    from concourse.masks import make_identity, make_upper_triangular

    f32 = mybir.dt.float32
    f32r = mybir.dt.float32r
    Exp = mybir.ActivationFunctionType.Exp

    nc = bacc.Bacc(
        "TRN2",
        target_bir_lowering=False,
        debug=False,
        enable_asserts=False,
        num_devices=NCORES,
    )

    xT = nc.dram_tensor("xT", [D, L], f32r, kind="ExternalInput").ap()
    wqT = nc.dram_tensor("wqT", [D, HPC * HS], f32r, kind="ExternalInput").ap()
    wout = nc.dram_tensor("wout", [HPC * HS, D], f32r, kind="ExternalInput").ap()
    bias = nc.dram_tensor("bias", [128, D // 128], f32, kind="ExternalInput").ap()
    zT = nc.dram_tensor("zT", [D, L], f32, kind="ExternalOutput").ap()

    xT_r = xT.rearrange("(c p) i -> p c i", p=128)  # [128, 6, L]
    zT_r = zT.rearrange("(c p) i -> c p i", p=128)  # [6, 128, L]

    with tile.TileContext(nc) as tc:
        with (
            tc.tile_pool(name="consts", bufs=1) as consts,
            tc.tile_pool(name="persist", bufs=1) as persist,
        ):
            # ---- constants ----
            wq_sb = consts.tile([128, 6, HPC * HS], f32r)
            nc.sync.dma_start(out=wq_sb, in_=wqT.rearrange("(c p) m -> p c m", p=128))
            wout0_sb = consts.tile([128, D], f32r)
            nc.sync.dma_start(out=wout0_sb, in_=wout[0:128, :])
            wout1_sb = consts.tile([64, D], f32r)
            nc.sync.dma_start(out=wout1_sb, in_=wout[128:192, :])
            bias_sb = consts.tile([128, 6], f32)
            nc.sync.dma_start(out=bias_sb, in_=bias)
            # 64x64 identity on both partition halves (transpose lhsT/rhs must
            # share a base partition; head 1 lives on partitions 64-127).
            # gpsimd can't write f32r, so build f32 then DVE-cast.
            ident_f = consts.tile([128, 64], f32)
            make_identity(nc, ident_f[0:64, :])
            make_identity(nc, ident_f[64:128, :])
            ident = consts.tile([128, 64], f32r)
            nc.vector.tensor_copy(out=ident, in_=ident_f)
            # keep mask[p, t] = 1.0 where t >= p else 0.0
            trimask = consts.tile([128, 128], f32)
            make_upper_triangular(nc, trimask, val=1.0, diag=True)

            # ---- persistent per-head state ----
            qt01 = persist.tile([128, L], f32r)  # Q^T heads 0,1 (rows 0-63 / 64-127)
            qt2 = persist.tile([64, L], f32r)  # Q^T head 2
            qn = persist.tile([128, 32, HPC, 65], f32r)  # Q natural + ones col
            ot01 = persist.tile([128, L], f32r)  # O^T heads 0,1
            ot2 = persist.tile([64, L], f32r)  # O^T head 2
            ones_f = consts.tile([128, 32, HPC, 1], f32)
            nc.vector.memset(ones_f, 1.0)
            nc.vector.tensor_copy(out=qn[:, :, :, 64:65], in_=ones_f)

            def qt_h(h, js, je, _t=(None,)):
                if h < 2:
                    return qt01[h * 64 : (h + 1) * 64, js:je]
                return qt2[:, js:je]

            def ot_h(h, js, je):
                if h < 2:
                    return ot01[h * 64 : (h + 1) * 64, js:je]
                return ot2[:, js:je]

            # ---- phase 1: Q^T projection ----
            with (
                tc.tile_pool(name="xin", bufs=2) as xin,
                tc.tile_pool(name="qps", bufs=2, space="PSUM") as qps,
            ):
                for ig in range(NIG):
                    i0 = ig * 512
                    xt = xin.tile([128, 6, 512], f32r, tag="xt")
                    nc.sync.dma_start(out=xt, in_=xT_r[:, :, i0 : i0 + 512])
                    q01 = qps.tile([128, 512], f32, tag="q01")
                    q2 = qps.tile([64, 512], f32, tag="q2")
                    for c in range(6):
                        nc.tensor.matmul(
                            q01,
                            lhsT=wq_sb[:, c, 0:128],
                            rhs=xt[:, c, :],
                            start=(c == 0),
                            stop=(c == 5),
                        )
                    for c in range(6):
                        nc.tensor.matmul(
                            q2,
                            lhsT=wq_sb[:, c, 128:192],
                            rhs=xt[:, c, :],
                            start=(c == 0),
                            stop=(c == 5),
                        )
                    nc.vector.tensor_copy(out=qt01[:, i0 : i0 + 512], in_=q01)
                    nc.vector.tensor_copy(out=qt2[:, i0 : i0 + 512], in_=q2)

                # ---- phase 1b: transpose Q^T -> Q natural ----
                with tc.tile_pool(name="tps", bufs=4, space="PSUM") as tps:
                    for jb in range(32):
                        for h in range(HPC):
                            tp = tps.tile([128, 64], f32r, tag="tp")
                            idn = ident[64:128, :] if h == 1 else ident[0:64, :]
                            nc.tensor.transpose(
                                tp, qt_h(h, jb * 128, (jb + 1) * 128), idn
                            )
                            nc.vector.tensor_copy(out=qn[:, jb, h, 0:64], in_=tp)

            # ---- phase 2: attention + output projection ----
            with (
                tc.tile_pool(name="scps", bufs=2, space="PSUM") as scps,
                tc.tile_pool(name="avps", bufs=2, space="PSUM") as avps,
                tc.tile_pool(name="ztps", bufs=2, space="PSUM") as ztps,
                tc.tile_pool(name="ptp", bufs=3) as ptp,
                tc.tile_pool(name="ztb", bufs=3) as ztb,
                tc.tile_pool(name="nrm", bufs=3) as nrm,
            ):
                for ig in range(NIG):
                    i0 = ig * 512
                    jb_max = 4 * (ig + 1)
                    for h in range(HPC):
                        av = avps.tile([65, 512], f32, tag="av")
                        for jg in range((jb_max + 1) // 2):
                            njb = min(2, jb_max - jg * 2)
                            sc = scps.tile([128, 1024], f32, tag="sc")
                            pt = ptp.tile([128, 1024], f32r, tag="pt")
                            for k in range(njb):
                                jb = jg * 2 + k
                                nc.tensor.matmul(
                                    sc[:, k * 512 : (k + 1) * 512],
                                    lhsT=qt_h(h, jb * 128, (jb + 1) * 128),
                                    rhs=qt_h(h, i0, i0 + 512),
                                    start=True,
                                    stop=True,
                                )
                            nc.scalar.activation(
                                out=pt[:, : njb * 512], in_=sc[:, : njb * 512], func=Exp
                            )
                            for k in range(njb):
                                jb = jg * 2 + k
                                r = jb - 4 * ig
                                sr = 128 * r if r > 0 else 0
                                if r >= 0:  # diagonal band: zero j > i
                                    band = slice(k * 512 + sr, k * 512 + sr + 128)
                                    nc.vector.tensor_mul(pt[:, band], pt[:, band], trimask)
                                nc.tensor.matmul(
                                    av[:, sr:512],
                                    lhsT=qn[:, jb, h, :],
                                    rhs=pt[:, k * 512 + sr : (k + 1) * 512],
                                    start=(jb == 0),
                                    stop=(jb == jb_max - 1),
                                    skip_group_check=True,
                                )
                        recip = nrm.tile([1, 512], f32, tag="recip")
                        nc.vector.reciprocal(recip, av[64:65, :])
                        rb = nrm.tile([64, 512], f32, tag="rb")
                        nc.gpsimd.partition_broadcast(rb, recip, channels=64)
                        nc.vector.tensor_mul(ot_h(h, i0, i0 + 512), av[0:64, :], rb)
                    for oc in range(6):
                        zt = ztps.tile([128, 512], f32, tag="zt")
                        nc.tensor.matmul(
                            zt,
                            lhsT=wout0_sb[:, oc * 128 : (oc + 1) * 128],
                            rhs=ot01[:, i0 : i0 + 512],
                            start=True,
                            stop=False,
                        )
                        nc.tensor.matmul(
                            zt,
                            lhsT=wout1_sb[:, oc * 128 : (oc + 1) * 128],
                            rhs=ot2[:, i0 : i0 + 512],
                            start=False,
                            stop=True,
                        )
                        zb = ztb.tile([128, 512], f32, tag="zb")
                        nc.vector.tensor_scalar_add(zb, zt, bias_sb[:, oc : oc + 1])
                        nc.sync.dma_start(out=zT_r[oc, :, i0 : i0 + 512], in_=zb)

    nc.compile()
    return nc


def _get_program():
    if "nc" not in _cached:
        _cached["nc"] = _build_program()
    return _cached["nc"]


def _make_in_maps(x, Wq, W_out, b_out):
    x = np.asarray(x, dtype=np.float32)
    Wq = np.asarray(Wq, dtype=np.float32)
    W_out = np.asarray(W_out, dtype=np.float32)
    b_out = np.asarray(b_out, dtype=np.float32)
    in_maps = []
    for c in range(NCORES):
        b = c // (NCORES // B)
        hg = c % (NCORES // B)
        h0 = hg * HPC
        xT = np.ascontiguousarray(x[b].T)  # [D, L]
        wq = Wq[h0 : h0 + HPC]  # [3, 64, D]
        wqT = np.ascontiguousarray(wq.transpose(2, 0, 1).reshape(D, HPC * HS) * SQS)
        wout = np.ascontiguousarray(W_out[:, h0 * HS : (h0 + HPC) * HS].T / SQS)
        bvec = b_out if hg == 0 else np.zeros_like(b_out)
        bias = np.ascontiguousarray(bvec.reshape(D // 128, 128).T)
        in_maps.append({"xT": xT, "wqT": wqT, "wout": wout, "bias": bias})
    return in_maps


def run(x, Wq, W_out, b_out, trace=False):
    from concourse.bass_utils import run_bass_kernel_spmd

    nc = _get_program()
    in_maps = _make_in_maps(x, Wq, W_out, b_out)
    res = run_bass_kernel_spmd(
        nc, in_maps, core_ids=list(range(NCORES)), trace=trace
    )
    partials = [r["zT"] for r in res.results]  # each [D, L]
    out = np.empty((B, L, D), dtype=np.float32)
    for b in range(B):
        g = NCORES // B
        acc = partials[b * g].copy()
        for c in range(b * g + 1, (b + 1) * g):
            acc += partials[c]
        out[b] = acc.T
    return out, res


def kernel(x, Wq, W_out, b_out):
    out, _ = run(x, Wq, W_out, b_out, trace=bool(int(os.environ.get("KERNEL_TRACE", "0"))))
    return out
